# revision 13
# baseline (speedup 1.0000x reference)
"""GNN edge-MLP kernel for 8 TRN2 NeuronCores.

reference:
    xs = x1[edge_index[0]]; xt = x2[edge_index[1]]
    h = relu(concat(xs, xt) @ W1 + b1); h = relu(h @ W2 + b2); out = h @ W3 + b3

Strategy (pure edge parallelism, no collectives):
  - Edges are sharded 8 ways (200k per core); node tables + weights replicated.
  - Host-side prep (numpy): cast node tables / weights to bf16, bucket each
    core's edges by (src_chunk, dst_chunk) over 4 node chunks of 25000 rows so
    the per-edge gather indices fit in int16 (dma_gather's index dtype), pad
    each bucket to a fixed capacity, and pre-wrap indices into dma_gather's
    [16 x n/16] interleaved layout.
  - Device kernel: per bucket, 2048-idx dma_gather calls (SWDGE,
    transpose=False — no scratch staging, half the descriptors of transpose
    mode) round-robined over 4 SWDGE queues so four DMA rings drain in
    parallel (the single-queue ring was the original bottleneck; transpose
    mode corrupts data when multiple queues stage concurrently). Rows land
    edge-major [128 edges, block, 128 feat]; the TensorEngine transposes each
    [128, 128] block via identity matmul into feat-major PSUM tiles, ACT/DVE
    copy them to SBUF bf16, then the MLP:
      L1: psum  = W1a.T @ xsT + W1b.T @ xtT      (two accumulating matmuls)
      s1 = relu(psum + b1)                        (ACT, cast to bf16)
      L2: psum2 = W2.T @ s1 ; s2 = relu(psum2+b2)
      L3: obank[row] = W3.T @ s2                  (M=1 matmul into a shared
                                                   psum bank, one row per tile)
    Output rows are flushed 32 tiles at a time (+ b3) to DRAM.
  - Host unpermutes bucket order back to the original edge order.
"""

import sys

sys.path.insert(0, "/opt/trn_rl_repo")

import functools

import ml_dtypes
import numpy as np

import concourse.bacc as bacc
import concourse.bass as bass
import concourse.mybir as mybir
import concourse.tile as tile
from concourse import library_config
from concourse.bass_utils import run_bass_kernel_spmd
from concourse.masks import make_identity

P = 128
D = 128
N_NODES = 100000
N_EDGES = 1600000
N_CORES = 8
E_CORE = N_EDGES // N_CORES  # 200000
NCHUNK = 4
CHUNK = N_NODES // NCHUNK  # 25000 rows per node chunk (int16-safe)
NBUCKET = NCHUNK * NCHUNK  # 16
MM_N = 512  # matmul moving free dim (one PSUM bank)
CAP = 13312  # per-bucket padded edge capacity (26 * 512); mean fill is 12500
FLUSH = 32  # tiles whose [1, 512] outputs accumulate in one psum bank

BF16 = mybir.dt.bfloat16
F32 = mybir.dt.float32
I16 = mybir.dt.int16
nbf = ml_dtypes.bfloat16


def _gcalls(cap):
    """Split a bucket's capacity into dma_gather call sizes (multiples of MM_N,
    capped at 4096 idxs; transpose=False gathers generate one descriptor per
    row, so four queues' in-flight calls fit the 65536-entry carveout)."""
    out = []
    base = 0
    while base < cap:
        sz = min(4096, cap - base)
        out.append((base, sz))
        base += sz
    return out


@functools.lru_cache(maxsize=2)
def build(cap=CAP):
    assert cap % MM_N == 0
    ntiles = NBUCKET * cap // MM_N
    nc = bacc.Bacc("TRN2", num_swdge_queues=4, dynamic_dma_scratch_size=65536)
    x1bf = nc.dram_tensor("x1bf", [N_NODES, D], BF16, kind="ExternalInput")
    x2bf = nc.dram_tensor("x2bf", [N_NODES, D], BF16, kind="ExternalInput")
    sidx = nc.dram_tensor("sidx", [NBUCKET, P, cap // 16], I16, kind="ExternalInput")
    didx = nc.dram_tensor("didx", [NBUCKET, P, cap // 16], I16, kind="ExternalInput")
    w1a = nc.dram_tensor("w1a", [D, D], BF16, kind="ExternalInput")
    w1b = nc.dram_tensor("w1b", [D, D], BF16, kind="ExternalInput")
    w2 = nc.dram_tensor("w2", [D, D], BF16, kind="ExternalInput")
    w3 = nc.dram_tensor("w3", [D, 1], BF16, kind="ExternalInput")
    b1 = nc.dram_tensor("b1", [D, 1], F32, kind="ExternalInput")
    b2 = nc.dram_tensor("b2", [D, 1], F32, kind="ExternalInput")
    b3 = nc.dram_tensor("b3", [P, 1], F32, kind="ExternalInput")
    out = nc.dram_tensor("out", [ntiles, MM_N], F32, kind="ExternalOutput")

    Relu = mybir.ActivationFunctionType.Relu
    Copy = mybir.ActivationFunctionType.Copy

    with tile.TileContext(nc) as tc:
        nc.gpsimd.load_library(library_config.mlp)
        with (
            tc.tile_pool(name="const", bufs=1) as cpool,
            tc.tile_pool(name="gxs", bufs=4) as gxs_pool,
            tc.tile_pool(name="gxt", bufs=4) as gxt_pool,
            tc.tile_pool(name="idx", bufs=8) as idx_pool,
            tc.tile_pool(name="act", bufs=3) as act_pool,
            tc.tile_pool(name="osb", bufs=2) as out_pool,
            tc.tile_pool(name="ph1", bufs=2, space="PSUM") as ph1,
            tc.tile_pool(name="ph2", bufs=1, space="PSUM") as ph2,
            tc.tile_pool(name="pob", bufs=1, space="PSUM") as pob,
            tc.tile_pool(name="ptx", bufs=2, space="PSUM") as ptx_pool,
            tc.tile_pool(name="pty", bufs=2, space="PSUM") as pty_pool,
        ):
            w1a_sb = cpool.tile([D, D], BF16, tag="w1a")
            w1b_sb = cpool.tile([D, D], BF16, tag="w1b")
            w2_sb = cpool.tile([D, D], BF16, tag="w2")
            w3_sb = cpool.tile([D, 1], BF16, tag="w3")
            b1_sb = cpool.tile([D, 1], F32, tag="b1")
            b2_sb = cpool.tile([D, 1], F32, tag="b2")
            b3_sb = cpool.tile([P, 1], F32, tag="b3")
            for sb, dr in [
                (w1a_sb, w1a), (w1b_sb, w1b), (w2_sb, w2), (w3_sb, w3),
                (b1_sb, b1), (b2_sb, b2), (b3_sb, b3),
            ]:
                nc.sync.dma_start(sb[:], dr[:])

            # w3 shifted into column m of slice m: L3's matmul for the m-th
            # tile of a flush group then lands its [1, 512] result on psum
            # partition m (accumulating zeros onto every other row).
            ident = cpool.tile([P, P], BF16, tag="ident")
            make_identity(nc, ident[:])

            w3m_sb = cpool.tile([P, FLUSH, D], BF16, tag="w3m")
            nc.vector.memset(w3m_sb[:], 0.0)
            for m in range(FLUSH):
                nc.vector.tensor_copy(w3m_sb[:, m, m : m + 1], w3_sb[:, 0:1])

            obank = None
            orow = 0
            oflushed = 0
            tile_no = 0
            qrr = [0]

            def flush(rows):
                nonlocal obank, orow, oflushed
                osb = out_pool.tile([P, MM_N], F32, tag="osb")
                nc.vector.tensor_scalar_add(
                    osb[:rows, :], obank[:rows, :], b3_sb[:rows, 0:1]
                )
                nc.sync.dma_start(out[oflushed : oflushed + rows, :], osb[:rows, :])
                oflushed += rows
                obank = None
                orow = 0

            for b in range(NBUCKET):
                ci, cj = b // NCHUNK, b % NCHUNK
                x1c = x1bf[ci * CHUNK : (ci + 1) * CHUNK, :]
                x2c = x2bf[cj * CHUNK : (cj + 1) * CHUNK, :]
                for base, gsz in _gcalls(cap):
                    c0 = base // 16
                    cols = gsz // 16
                    sidx_sb = idx_pool.tile([P, cols], I16, tag="sidx")
                    didx_sb = idx_pool.tile([P, cols], I16, tag="didx")
                    nc.sync.dma_start(sidx_sb[:], sidx[b, :, c0 : c0 + cols])
                    nc.sync.dma_start(didx_sb[:], didx[b, :, c0 : c0 + cols])
                    xs = gxs_pool.tile([P, gsz // P, D], BF16, tag="xs")
                    xt = gxt_pool.tile([P, gsz // P, D], BF16, tag="xt")
                    nc.gpsimd.dma_gather(
                        xs[:], x1c, sidx_sb[:], gsz, gsz, D,
                        transpose=False, single_packet=False,
                        queue_num=qrr[0] % 4,
                    )
                    qrr[0] += 1
                    nc.gpsimd.dma_gather(
                        xt[:], x2c, didx_sb[:], gsz, gsz, D,
                        transpose=False, single_packet=False,
                        queue_num=qrr[0] % 4,
                    )
                    qrr[0] += 1
                    for k in range(gsz // MM_N):
                        ptxs = ptx_pool.tile([P, MM_N], BF16, tag="ptxs")
                        ptxt = pty_pool.tile([P, MM_N], BF16, tag="ptxt")
                        for i in range(4):
                            j = k * 4 + i
                            nc.tensor.transpose(
                                ptxs[:, i * P : (i + 1) * P], xs[:, j, :], ident[:]
                            )
                        for i in range(4):
                            j = k * 4 + i
                            nc.tensor.transpose(
                                ptxt[:, i * P : (i + 1) * P], xt[:, j, :], ident[:]
                            )
                        sxs = act_pool.tile([P, MM_N], BF16, tag="sxs")
                        sxt = act_pool.tile([P, MM_N], BF16, tag="sxt")
                        nc.vector.tensor_copy(sxs[:], ptxs[:])
                        nc.vector.tensor_copy(sxt[:], ptxt[:])
                        h1 = ph1.tile([P, MM_N], F32, tag="h1")
                        nc.tensor.matmul(
                            h1[:], w1a_sb[:], sxs[:], start=True, stop=False
                        )
                        nc.tensor.matmul(
                            h1[:], w1b_sb[:], sxt[:], start=False, stop=True
                        )
                        s1 = act_pool.tile([P, MM_N], BF16, tag="s1")
                        nc.scalar.activation(s1[:], h1[:], Relu, bias=b1_sb[:, 0:1])
                        h2 = ph2.tile([P, MM_N], F32, tag="h2")
                        nc.tensor.matmul(h2[:], w2_sb[:], s1[:], start=True, stop=True)
                        s2 = act_pool.tile([P, MM_N], BF16, tag="s2")
                        nc.scalar.activation(s2[:], h2[:], Relu, bias=b2_sb[:, 0:1])
                        if obank is None:
                            obank = pob.tile([P, MM_N], F32, tag="ob")
                        grp = min(FLUSH, ntiles - (tile_no - orow))
                        nc.tensor.matmul(
                            obank[:], w3m_sb[:, orow, :], s2[:],
                            start=(orow == 0), stop=(orow == grp - 1),
                        )
                        orow += 1
                        tile_no += 1
                        if orow == grp:
                            flush(grp)
            if orow:
                flush(orow)
    nc.compile()
    return nc


def _wrap_idx(arr, cap):
    """[NBUCKET, cap] int16 -> dma_gather layout [NBUCKET, 128, cap // 16]
    (idx i lives at partition i % 16, column i // 16; replicated 8x)."""
    w = arr.reshape(NBUCKET, cap // 16, 16).transpose(0, 2, 1)
    return np.tile(w, (1, 8, 1)).copy()


def _prep_core(src, dst, cap):
    """Bucket one core's edges by (src chunk, dst chunk). Returns the wrapped
    int16 local-index tensors, the bucket-grouped edge order, and counts."""
    bucket = (src // CHUNK) * NCHUNK + dst // CHUNK
    order = np.argsort(bucket, kind="stable")
    counts = np.bincount(bucket, minlength=NBUCKET)
    sloc = np.zeros(NBUCKET * cap, np.int16)
    dloc = np.zeros(NBUCKET * cap, np.int16)
    pos = 0
    for b in range(NBUCKET):
        grp = order[pos : pos + counts[b]]
        pos += counts[b]
        sloc[b * cap : b * cap + counts[b]] = src[grp] - (b // NCHUNK) * CHUNK
        dloc[b * cap : b * cap + counts[b]] = dst[grp] - (b % NCHUNK) * CHUNK
    return (
        _wrap_idx(sloc.reshape(NBUCKET, cap), cap),
        _wrap_idx(dloc.reshape(NBUCKET, cap), cap),
        order,
        counts,
    )


def kernel(x1, x2, edge_index, W1, b1, W2, b2, W3, b3, _trace=False):
    x1 = np.asarray(x1)
    x2 = np.asarray(x2)
    edge_index = np.asarray(edge_index)
    n_edges = edge_index.shape[1]
    assert x1.shape == (N_NODES, D) and x2.shape == (N_NODES, D)
    assert n_edges % N_CORES == 0
    e_core = n_edges // N_CORES

    x1bf = x1.astype(nbf)
    x2bf = x2.astype(nbf)
    W1 = np.asarray(W1, np.float32)
    w1a = W1[:D].astype(nbf)
    w1b = W1[D:].astype(nbf)
    w2 = np.asarray(W2, np.float32).astype(nbf)
    w3 = np.asarray(W3, np.float32).astype(nbf)
    b1c = np.asarray(b1, np.float32).reshape(D, 1)
    b2c = np.asarray(b2, np.float32).reshape(D, 1)
    b3c = np.full((P, 1), np.float32(np.asarray(b3).reshape(-1)[0]), np.float32)

    src_all = np.ascontiguousarray(edge_index[0]).astype(np.int64)
    dst_all = np.ascontiguousarray(edge_index[1]).astype(np.int64)

    preps = []
    max_count = 0
    for c in range(N_CORES):
        sl = slice(c * e_core, (c + 1) * e_core)
        bucket_max = 0
        src = src_all[sl]
        dst = dst_all[sl]
        counts = np.bincount((src // CHUNK) * NCHUNK + dst // CHUNK, minlength=NBUCKET)
        bucket_max = int(counts.max())
        max_count = max(max_count, bucket_max)
        preps.append((src, dst))
    cap = CAP if max_count <= CAP else -(-max_count // MM_N) * MM_N

    nc = build(cap)
    in_maps = []
    orders = []
    countss = []
    for src, dst in preps:
        sidx, didx, order, counts = _prep_core(src, dst, cap)
        orders.append(order)
        countss.append(counts)
        in_maps.append(
            {
                "x1bf": x1bf, "x2bf": x2bf, "sidx": sidx, "didx": didx,
                "w1a": w1a, "w1b": w1b, "w2": w2, "w3": w3,
                "b1": b1c, "b2": b2c, "b3": b3c,
            }
        )

    res = run_bass_kernel_spmd(
        nc, in_maps, core_ids=list(range(N_CORES)), trace=_trace
    )

    result = np.empty((n_edges,), np.float32)
    for c in range(N_CORES):
        flat = res.results[c]["out"].reshape(NBUCKET, cap)
        vals = np.concatenate(
            [flat[b, : countss[c][b]] for b in range(NBUCKET)]
        )
        r = np.empty((e_core,), np.float32)
        r[orders[c]] = vals
        result[c * e_core : (c + 1) * e_core] = r

    if _trace:
        kernel.last_exec_time_ns = res.exec_time_ns
        kernel.last_res = res
    return result.reshape(n_edges, 1)



# revision 15
# speedup vs baseline: 1.2270x; 1.2270x over previous
"""GNN edge-MLP kernel for 8 TRN2 NeuronCores.

reference:
    xs = x1[edge_index[0]]; xt = x2[edge_index[1]]
    h = relu(concat(xs, xt) @ W1 + b1); h = relu(h @ W2 + b2); out = h @ W3 + b3

Strategy (pure edge parallelism, no collectives):
  - Edges are sharded 8 ways (200k per core); node tables + weights replicated.
  - Host-side prep (numpy): cast node tables / weights to bf16, bucket each
    core's edges by (src_chunk, dst_chunk) over 4 node chunks of 25000 rows so
    the per-edge gather indices fit in int16 (dma_gather's index dtype), pad
    each bucket to a fixed capacity, and pre-wrap indices into dma_gather's
    [16 x n/16] interleaved layout.
  - Device kernel: per bucket, 2048-idx dma_gather calls (SWDGE,
    transpose=False — no scratch staging, half the descriptors of transpose
    mode) round-robined over 4 SWDGE queues so four DMA rings drain in
    parallel (the single-queue ring was the original bottleneck; transpose
    mode corrupts data when multiple queues stage concurrently). Rows land
    edge-major [128 edges, block, 128 feat]; the TensorEngine transposes each
    [128, 128] block via identity matmul into feat-major PSUM tiles, ACT/DVE
    copy them to SBUF bf16, then the MLP:
      L1: psum  = W1a.T @ xsT + W1b.T @ xtT      (two accumulating matmuls)
      s1 = relu(psum + b1)                        (ACT, cast to bf16)
      L2: psum2 = W2.T @ s1 ; s2 = relu(psum2+b2)
      L3: obank[row] = W3.T @ s2                  (M=1 matmul into a shared
                                                   psum bank, one row per tile)
    Output rows are flushed 32 tiles at a time (+ b3) to DRAM.
  - Host unpermutes bucket order back to the original edge order.
"""

import sys

sys.path.insert(0, "/opt/trn_rl_repo")

import functools

import ml_dtypes
import numpy as np

import concourse.bacc as bacc
import concourse.bass as bass
import concourse.mybir as mybir
import concourse.tile as tile
from concourse import library_config
from concourse.bass_utils import run_bass_kernel_spmd
from concourse.masks import make_identity

P = 128
D = 128
N_NODES = 100000
N_EDGES = 1600000
N_CORES = 8
E_CORE = N_EDGES // N_CORES  # 200000
NCHUNK = 4
CHUNK = N_NODES // NCHUNK  # 25000 rows per node chunk (int16-safe)
NBUCKET = NCHUNK * NCHUNK  # 16
MM_N = 512  # matmul moving free dim (one PSUM bank)
CAP = 13312  # per-bucket padded edge capacity (26 * 512); mean fill is 12500
FLUSH = 32  # tiles whose [1, 512] outputs accumulate in one psum bank

BF16 = mybir.dt.bfloat16
F32 = mybir.dt.float32
I16 = mybir.dt.int16
nbf = ml_dtypes.bfloat16


def _gcalls(cap):
    """Split a bucket's capacity into dma_gather call sizes (multiples of MM_N,
    capped at 2048 idxs to keep the four SWDGE queues' rings finely
    pipelined — 4096-idx calls measured ~25% slower end-to-end)."""
    out = []
    base = 0
    while base < cap:
        sz = min(2048, cap - base)
        out.append((base, sz))
        base += sz
    return out


@functools.lru_cache(maxsize=2)
def build(cap=CAP):
    assert cap % MM_N == 0
    ntiles = NBUCKET * cap // MM_N
    nc = bacc.Bacc("TRN2", num_swdge_queues=4, dynamic_dma_scratch_size=65536)
    x1bf = nc.dram_tensor("x1bf", [N_NODES, D], BF16, kind="ExternalInput")
    x2bf = nc.dram_tensor("x2bf", [N_NODES, D], BF16, kind="ExternalInput")
    sidx = nc.dram_tensor("sidx", [NBUCKET, P, cap // 16], I16, kind="ExternalInput")
    didx = nc.dram_tensor("didx", [NBUCKET, P, cap // 16], I16, kind="ExternalInput")
    w1a = nc.dram_tensor("w1a", [D, D], BF16, kind="ExternalInput")
    w1b = nc.dram_tensor("w1b", [D, D], BF16, kind="ExternalInput")
    w2 = nc.dram_tensor("w2", [D, D], BF16, kind="ExternalInput")
    w3 = nc.dram_tensor("w3", [D, 1], BF16, kind="ExternalInput")
    b1 = nc.dram_tensor("b1", [D, 1], F32, kind="ExternalInput")
    b2 = nc.dram_tensor("b2", [D, 1], F32, kind="ExternalInput")
    b3 = nc.dram_tensor("b3", [P, 1], F32, kind="ExternalInput")
    out = nc.dram_tensor("out", [ntiles, MM_N], F32, kind="ExternalOutput")

    Relu = mybir.ActivationFunctionType.Relu
    Copy = mybir.ActivationFunctionType.Copy

    with tile.TileContext(nc) as tc:
        nc.gpsimd.load_library(library_config.mlp)
        with (
            tc.tile_pool(name="const", bufs=1) as cpool,
            tc.tile_pool(name="gxs", bufs=4) as gxs_pool,
            tc.tile_pool(name="gxt", bufs=4) as gxt_pool,
            tc.tile_pool(name="idx", bufs=8) as idx_pool,
            tc.tile_pool(name="act", bufs=3) as act_pool,
            tc.tile_pool(name="osb", bufs=2) as out_pool,
            tc.tile_pool(name="ph1", bufs=1, space="PSUM") as ph1,
            tc.tile_pool(name="ph2", bufs=1, space="PSUM") as ph2,
            tc.tile_pool(name="pob", bufs=2, space="PSUM") as pob,
            tc.tile_pool(name="ptx", bufs=2, space="PSUM") as ptx_pool,
            tc.tile_pool(name="pty", bufs=2, space="PSUM") as pty_pool,
        ):
            w1a_sb = cpool.tile([D, D], BF16, tag="w1a")
            w1b_sb = cpool.tile([D, D], BF16, tag="w1b")
            w2_sb = cpool.tile([D, D], BF16, tag="w2")
            w3_sb = cpool.tile([D, 1], BF16, tag="w3")
            b1_sb = cpool.tile([D, 1], F32, tag="b1")
            b2_sb = cpool.tile([D, 1], F32, tag="b2")
            b3_sb = cpool.tile([P, 1], F32, tag="b3")
            for sb, dr in [
                (w1a_sb, w1a), (w1b_sb, w1b), (w2_sb, w2), (w3_sb, w3),
                (b1_sb, b1), (b2_sb, b2), (b3_sb, b3),
            ]:
                nc.sync.dma_start(sb[:], dr[:])

            # w3 shifted into column m of slice m: L3's matmul for the m-th
            # tile of a flush group then lands its [1, 512] result on psum
            # partition m (accumulating zeros onto every other row).
            ident = cpool.tile([P, P], BF16, tag="ident")
            make_identity(nc, ident[:])

            w3m_sb = cpool.tile([P, FLUSH, D], BF16, tag="w3m")
            nc.vector.memset(w3m_sb[:], 0.0)
            for m in range(FLUSH):
                nc.vector.tensor_copy(w3m_sb[:, m, m : m + 1], w3_sb[:, 0:1])

            obank = None
            orow = 0
            oflushed = 0
            tile_no = 0
            qrr = [0]

            def flush(rows):
                nonlocal obank, orow, oflushed
                osb = out_pool.tile([P, MM_N], F32, tag="osb")
                nc.vector.tensor_scalar_add(
                    osb[:rows, :], obank[:rows, :], b3_sb[:rows, 0:1]
                )
                nc.sync.dma_start(out[oflushed : oflushed + rows, :], osb[:rows, :])
                oflushed += rows
                obank = None
                orow = 0

            for b in range(NBUCKET):
                ci, cj = b // NCHUNK, b % NCHUNK
                x1c = x1bf[ci * CHUNK : (ci + 1) * CHUNK, :]
                x2c = x2bf[cj * CHUNK : (cj + 1) * CHUNK, :]
                for base, gsz in _gcalls(cap):
                    c0 = base // 16
                    cols = gsz // 16
                    sidx_sb = idx_pool.tile([P, cols], I16, tag="sidx")
                    didx_sb = idx_pool.tile([P, cols], I16, tag="didx")
                    nc.sync.dma_start(sidx_sb[:], sidx[b, :, c0 : c0 + cols])
                    nc.sync.dma_start(didx_sb[:], didx[b, :, c0 : c0 + cols])
                    xs = gxs_pool.tile([P, gsz // P, D], BF16, tag="xs")
                    xt = gxt_pool.tile([P, gsz // P, D], BF16, tag="xt")
                    nc.gpsimd.dma_gather(
                        xs[:], x1c, sidx_sb[:], gsz, gsz, D,
                        transpose=False, single_packet=False,
                        queue_num=qrr[0] % 4,
                    )
                    qrr[0] += 1
                    nc.gpsimd.dma_gather(
                        xt[:], x2c, didx_sb[:], gsz, gsz, D,
                        transpose=False, single_packet=False,
                        queue_num=qrr[0] % 4,
                    )
                    qrr[0] += 1
                    for k in range(gsz // MM_N):
                        ptxs = ptx_pool.tile([P, MM_N], BF16, tag="ptxs")
                        ptxt = pty_pool.tile([P, MM_N], BF16, tag="ptxt")
                        for i in range(4):
                            j = k * 4 + i
                            nc.tensor.transpose(
                                ptxs[:, i * P : (i + 1) * P], xs[:, j, :], ident[:]
                            )
                        for i in range(4):
                            j = k * 4 + i
                            nc.tensor.transpose(
                                ptxt[:, i * P : (i + 1) * P], xt[:, j, :], ident[:]
                            )
                        sxs = act_pool.tile([P, MM_N], BF16, tag="sxs")
                        sxt = act_pool.tile([P, MM_N], BF16, tag="sxt")
                        nc.scalar.activation(sxs[:], ptxs[:], Copy)
                        nc.vector.tensor_copy(sxt[:], ptxt[:])
                        h1 = ph1.tile([P, MM_N], F32, tag="h1")
                        nc.tensor.matmul(
                            h1[:], w1a_sb[:], sxs[:], start=True, stop=False
                        )
                        nc.tensor.matmul(
                            h1[:], w1b_sb[:], sxt[:], start=False, stop=True
                        )
                        s1 = act_pool.tile([P, MM_N], BF16, tag="s1")
                        nc.scalar.activation(s1[:], h1[:], Relu, bias=b1_sb[:, 0:1])
                        h2 = ph2.tile([P, MM_N], F32, tag="h2")
                        nc.tensor.matmul(h2[:], w2_sb[:], s1[:], start=True, stop=True)
                        s2 = act_pool.tile([P, MM_N], BF16, tag="s2")
                        nc.vector.tensor_scalar(
                            s2[:], h2[:], b2_sb[:, 0:1], 0.0,
                            mybir.AluOpType.add, mybir.AluOpType.max,
                        )
                        if obank is None:
                            obank = pob.tile([P, MM_N], F32, tag="ob")
                        grp = min(FLUSH, ntiles - (tile_no - orow))
                        nc.tensor.matmul(
                            obank[:], w3m_sb[:, orow, :], s2[:],
                            start=(orow == 0), stop=(orow == grp - 1),
                        )
                        orow += 1
                        tile_no += 1
                        if orow == grp:
                            flush(grp)
            if orow:
                flush(orow)
    nc.compile()
    return nc


def _wrap_idx(arr, cap):
    """[NBUCKET, cap] int16 -> dma_gather layout [NBUCKET, 128, cap // 16]
    (idx i lives at partition i % 16, column i // 16; replicated 8x)."""
    w = arr.reshape(NBUCKET, cap // 16, 16).transpose(0, 2, 1)
    return np.tile(w, (1, 8, 1)).copy()


def _prep_core(src, dst, cap):
    """Bucket one core's edges by (src chunk, dst chunk). Returns the wrapped
    int16 local-index tensors, the bucket-grouped edge order, and counts."""
    bucket = (src // CHUNK) * NCHUNK + dst // CHUNK
    order = np.argsort(bucket, kind="stable")
    counts = np.bincount(bucket, minlength=NBUCKET)
    sloc = np.zeros(NBUCKET * cap, np.int16)
    dloc = np.zeros(NBUCKET * cap, np.int16)
    pos = 0
    for b in range(NBUCKET):
        grp = order[pos : pos + counts[b]]
        pos += counts[b]
        sloc[b * cap : b * cap + counts[b]] = src[grp] - (b // NCHUNK) * CHUNK
        dloc[b * cap : b * cap + counts[b]] = dst[grp] - (b % NCHUNK) * CHUNK
    return (
        _wrap_idx(sloc.reshape(NBUCKET, cap), cap),
        _wrap_idx(dloc.reshape(NBUCKET, cap), cap),
        order,
        counts,
    )


def kernel(x1, x2, edge_index, W1, b1, W2, b2, W3, b3, _trace=False):
    x1 = np.asarray(x1)
    x2 = np.asarray(x2)
    edge_index = np.asarray(edge_index)
    n_edges = edge_index.shape[1]
    assert x1.shape == (N_NODES, D) and x2.shape == (N_NODES, D)
    assert n_edges % N_CORES == 0
    e_core = n_edges // N_CORES

    x1bf = x1.astype(nbf)
    x2bf = x2.astype(nbf)
    W1 = np.asarray(W1, np.float32)
    w1a = W1[:D].astype(nbf)
    w1b = W1[D:].astype(nbf)
    w2 = np.asarray(W2, np.float32).astype(nbf)
    w3 = np.asarray(W3, np.float32).astype(nbf)
    b1c = np.asarray(b1, np.float32).reshape(D, 1)
    b2c = np.asarray(b2, np.float32).reshape(D, 1)
    b3c = np.full((P, 1), np.float32(np.asarray(b3).reshape(-1)[0]), np.float32)

    src_all = np.ascontiguousarray(edge_index[0]).astype(np.int64)
    dst_all = np.ascontiguousarray(edge_index[1]).astype(np.int64)

    preps = []
    max_count = 0
    for c in range(N_CORES):
        sl = slice(c * e_core, (c + 1) * e_core)
        bucket_max = 0
        src = src_all[sl]
        dst = dst_all[sl]
        counts = np.bincount((src // CHUNK) * NCHUNK + dst // CHUNK, minlength=NBUCKET)
        bucket_max = int(counts.max())
        max_count = max(max_count, bucket_max)
        preps.append((src, dst))
    cap = CAP if max_count <= CAP else -(-max_count // MM_N) * MM_N

    nc = build(cap)
    in_maps = []
    orders = []
    countss = []
    for src, dst in preps:
        sidx, didx, order, counts = _prep_core(src, dst, cap)
        orders.append(order)
        countss.append(counts)
        in_maps.append(
            {
                "x1bf": x1bf, "x2bf": x2bf, "sidx": sidx, "didx": didx,
                "w1a": w1a, "w1b": w1b, "w2": w2, "w3": w3,
                "b1": b1c, "b2": b2c, "b3": b3c,
            }
        )

    res = run_bass_kernel_spmd(
        nc, in_maps, core_ids=list(range(N_CORES)), trace=_trace
    )

    result = np.empty((n_edges,), np.float32)
    for c in range(N_CORES):
        flat = res.results[c]["out"].reshape(NBUCKET, cap)
        vals = np.concatenate(
            [flat[b, : countss[c][b]] for b in range(NBUCKET)]
        )
        r = np.empty((e_core,), np.float32)
        r[orders[c]] = vals
        result[c * e_core : (c + 1) * e_core] = r

    if _trace:
        kernel.last_exec_time_ns = res.exec_time_ns
        kernel.last_res = res
    return result.reshape(n_edges, 1)

